# revision 4
# baseline (speedup 1.0000x reference)
"""Causal single-head attention block on 8 TRN2 NeuronCores.

Reference: Q=x@Wq, K=x@Wk, V=x@Wv; S=Q@K^T (no pre-softmax scaling);
causal mask; P=softmax(S); out=(P@V)/sqrt(64).
Shapes: x [4, 2048, 1024] f32, W* [1024, 64] f32 -> out [4, 2048, 64].

Sharding: 8 cores = 4 batches x 2 interleaved query-tile sets.
Core (b, jj) owns 8 query tiles of 128 rows:
  jj=0: g = {0,2,4,6,9,11,13,15},  jj=1: g = {1,3,5,7,8,10,12,14}
Both sets have equal causal work at 128-key granularity. Host permutes
x[b]^T (fp16) into "slots": 0..7 = own query tiles ascending, 8..15 =
complementary tiles ascending, so the device program is SPMD. Causality
= one shared triangular mask on the diagonal block + a per-core 0/1
scalar folded into a pre-scaled boundary V slot ("vz").

On-chip dataflow per core (x^T resident in SBUF, fp16):
  own slots:  psum[kq, t] = [Wk|Wq]^T @ x^T   (fused, 1 cy/col)
              psum[t, v]  = x^T-tile.T @ (Wv/8)  (natural V)
  comp slots: psum[kv, t] = [Wk|Wv/8]^T @ x^T (fused: V^T rides free)
              V^T -> V via 64-col PE transpose (identity operand)
  S^T tile [t,q] = K^T-slot.T @ Q^T-tile -> exp on ACT -> E^T bf16
  E^T diag block *= tri (DVE, 16-bit 2x mode)
  out psum [q, 65] += E^T-block.T @ [V | 1]  (ones col = row sums)
  raw psum [64 cols + rowsum] shipped to DRAM; host divides.

Engine split: PE matmuls; ACT exp (1-4 block chunks); DVE kt/qt/vt
pair copies + tri + svec; Pool(GpSimd) vv/vz/po copies; SP all DMAs.
Two tiny warmup matmuls set pe_busy_start early (later matmuls are
costed against that timestamp; full clock ~3us after it).
"""

import sys

import numpy as np
import ml_dtypes

try:  # concourse ships in the TRN container; fall back to its known path
    import concourse  # noqa: F401
except ImportError:
    sys.path.insert(0, "/opt/trn_rl_repo")

B, T, C, DK = 4, 2048, 1024, 64
NLI = 8          # query tiles per core
NSLOT = 16       # key tiles (slots) per batch

_CACHE = {}


def _build():
    import concourse.bacc as bacc
    import concourse.tile as tile
    import concourse.mybir as mybir

    f32 = mybir.dt.float32
    f16 = mybir.dt.float16
    bf16 = mybir.dt.bfloat16
    EXP = mybir.ActivationFunctionType.Exp

    nc = bacc.Bacc("TRN2", target_bir_lowering=False, debug=False,
                   enable_asserts=False, num_devices=8)

    xt_d = nc.dram_tensor("xt", [128, NSLOT, 1024], f16,
                          kind="ExternalInput").ap()
    w_d = nc.dram_tensor("w", [128, 8, 3, 64], f16,
                         kind="ExternalInput").ap()
    idt_d = nc.dram_tensor("idt", [64, 64], f16, kind="ExternalInput").ap()
    msk_d = nc.dram_tensor("msk", [128, 136], bf16,
                           kind="ExternalInput").ap()
    y_d = nc.dram_tensor("y", [128, NLI, DK + 1], f32,
                         kind="ExternalOutput").ap()

    with tile.TileContext(nc) as tc:
        with (
            tc.tile_pool(name="persist", bufs=1) as pp,
            tc.tile_pool(name="pmix", bufs=2, space="PSUM") as pmx,
            tc.tile_pool(name="pst", bufs=4, space="PSUM") as pst,
            tc.tile_pool(name="pout", bufs=2, space="PSUM") as pou,
        ):
            xt = pp.tile([128, NSLOT, 1024], f16, tag="xt", name="xt")
            w = pp.tile([128, 8, 3, 64], f16, tag="w", name="w")
            idt = pp.tile([64, 64], f16, tag="idt", name="idt")
            msk = pp.tile([128, 136], bf16, tag="msk", name="msk")
            tri = msk[:, 0:128]
            svec = pp.tile([128, NLI], f32, tag="svec", name="svec")
            kt = pp.tile([64, NSLOT, 128], f16, tag="kt", name="kt")
            qt = pp.tile([64, NLI, 128], f16, tag="qt", name="qt")
            vt = pp.tile([64, NLI, 128], f16, tag="vt", name="vt")
            vv = pp.tile([128, NSLOT, DK + 1], bf16, tag="vv", name="vv")
            vz = pp.tile([128, NLI, DK + 1], bf16, tag="vz", name="vz")
            po_sb = pp.tile([128, NLI, DK + 1], f32, tag="posb", name="posb")
            scr = pp.tile([128, 128], f16, tag="scr", name="scr")
            E = [pp.tile([128, (2 * li + 2) * 128], bf16, tag=f"E{li}",
                         name=f"E{li}") for li in range(NLI)]

            nc.vector.memset(scr, 0.0)
            nc.vector.memset(vv[:, :, DK:DK + 1], 1.0)

            # ---- DMA program (all on SP; transfers serialize in this
            # order on the shared DMA engines) ----
            nc.sync.dma_start(w[:, 0:5], w_d[:, 0:5])
            nc.sync.dma_start(xt[:, 0, 0:512], xt_d[:, 0, 0:512])
            nc.sync.dma_start(w[:, 5:8], w_d[:, 5:8])
            nc.sync.dma_start(xt[:, 0, 512:1024], xt_d[:, 0, 512:1024])
            nc.sync.dma_start(xt[:, 1, :], xt_d[:, 1, :])
            nc.sync.dma_start(xt[:, 2:4, :], xt_d[:, 2:4, :])
            nc.sync.dma_start(xt[:, 4:6, :], xt_d[:, 4:6, :])
            nc.sync.dma_start(msk, msk_d)
            nc.sync.dma_start(idt, idt_d)
            nc.sync.dma_start(xt[:, 6:8, :], xt_d[:, 6:8, :])
            nc.sync.dma_start(xt[:, 8:10, :], xt_d[:, 8:10, :])
            nc.sync.dma_start(xt[:, 10:12, :], xt_d[:, 10:12, :])
            nc.sync.dma_start(xt[:, 12:14, :], xt_d[:, 12:14, :])
            nc.sync.dma_start(xt[:, 14, :], xt_d[:, 14, :])
            nc.sync.dma_start(xt[:, 15, 0:512], xt_d[:, 15, 0:512])
            nc.sync.dma_start(xt[:, 15, 512:1024], xt_d[:, 15, 512:1024])

            # ---- minimal PE warmup (sets pe_busy_start ~1.2us) ----
            for _ in range(2):
                pw = pmx.tile([128, 2, 128], f32, tag="pmix", name="pw")
                nc.tensor.matmul(pw[:, 0, 0:2], scr, scr[:, 0:2],
                                 start=True, stop=True)

            ps_of = {}

            def kq_own(s0, nslots, chunks, start, stop):
                """[Wk|Wq] fused projection for own slots [s0, s0+n)."""
                key = ("kq", s0)
                if key not in ps_of:
                    ps_of[key] = pmx.tile([128, 2, 128], f32, tag="pmix",
                                          name=f"pkq{s0}")
                ps = ps_of[key]
                for i in range(nslots):
                    s = s0 + i
                    for ch in chunks:
                        nc.tensor.matmul(
                            ps[:, i, :],
                            w[:, ch, 0:2, :],
                            xt[:, s, ch * 128:(ch + 1) * 128],
                            start=(start and ch == chunks[0]),
                            stop=(stop and ch == chunks[-1]),
                        )
                if stop:
                    nc.vector.tensor_copy(kt[:, s0:s0 + nslots, :],
                                          ps[0:64, 0:nslots, :])
                    nc.vector.tensor_copy(qt[:, s0:s0 + nslots, :],
                                          ps[64:128, 0:nslots, :])

            def v_own(s0, nslots):
                """Natural V (pre-scaled 1/8) for own slots [s0, s0+n)."""
                ps = pmx.tile([128, 2, 128], f32, tag="pmix", name=f"pv{s0}")
                for i in range(nslots):
                    s = s0 + i
                    for ch in range(8):
                        nc.tensor.matmul(
                            ps[:, i, 0:DK],
                            xt[:, s, ch * 128:(ch + 1) * 128],
                            w[:, ch, 2, :],
                            start=(ch == 0), stop=(ch == 7),
                        )
                nc.gpsimd.tensor_copy(vv[:, s0:s0 + nslots, 0:DK],
                                      ps[:, 0:nslots, 0:DK])

            def kv_comp(s0, nslots, chunks=tuple(range(8)), start=True,
                        stop=True):
                """[Wk|Wv/8] fused projection for comp slots [s0, s0+n);
                V^T -> vt staging -> PE transpose -> vv natural."""
                key = ("kv", s0)
                if key not in ps_of:
                    ps_of[key] = pmx.tile([128, 2, 128], f32, tag="pmix",
                                          name=f"pkv{s0}")
                ps = ps_of[key]
                for i in range(nslots):
                    s = s0 + i
                    for ch in chunks:
                        nc.tensor.matmul(
                            ps[:, i, :],
                            w[:, ch, 0:3:2, :],
                            xt[:, s, ch * 128:(ch + 1) * 128],
                            start=(start and ch == chunks[0]),
                            stop=(stop and ch == chunks[-1]),
                        )
                if not stop:
                    return
                k0 = s0 - 8
                nc.vector.tensor_copy(kt[:, s0:s0 + nslots, :],
                                      ps[0:64, 0:nslots, :])
                nc.vector.tensor_copy(vt[:, k0:k0 + nslots, :],
                                      ps[64:128, 0:nslots, :])
                pt = pmx.tile([128, 2, DK], f16, tag="pmix", name=f"pt{s0}")
                for i in range(nslots):
                    nc.tensor.transpose(pt[:, i, :], vt[:, k0 + i, :], idt)
                nc.gpsimd.tensor_copy(vv[:, s0:s0 + nslots, 0:DK],
                                      pt[:, 0:nslots, :])

            def s_exp(li, blocks, tag=""):
                """S^T then exp for E[li] col blocks `blocks` (block j:
                key slot j if j<=li else 8+(j-li-1); q = slot li)."""
                nb = len(blocks)
                ps = pst.tile([128, 512], f32, tag="pst",
                              name=f"ps{li}{tag}")
                for i, j in enumerate(blocks):
                    s = j if j <= li else 8 + (j - li - 1)
                    nc.tensor.matmul(
                        ps[:, i * 128:(i + 1) * 128],
                        kt[:, s, :],
                        qt[:, li, :],
                        start=True, stop=True,
                    )
                j0 = blocks[0]
                nc.scalar.activation(
                    E[li][:, j0 * 128:(j0 + nb) * 128], ps[:, 0:nb * 128],
                    EXP)

            def tri_mul(li):
                # diagonal block (j = li): shared triangular mask on DVE
                nc.vector.tensor_mul(
                    E[li][:, li * 128:(li + 1) * 128],
                    E[li][:, li * 128:(li + 1) * 128], tri)

            def vz_make(li):
                # boundary V slot pre-scaled by the per-core 0/1 scalar
                nc.gpsimd.tensor_scalar_mul(
                    vz[:, li, :], vv[:, 8 + li, :], svec[:, li:li + 1])

            po_t = {}

            def pv_mm(li, blocks, start, stop):
                if li not in po_t:
                    po_t[li] = pou.tile([128, DK + 1], f32, tag="pout",
                                        name=f"po{li}")
                po = po_t[li]
                last = blocks[-1]
                for j in blocks:
                    if j == 2 * li + 1:
                        rhs = vz[:, li, :]
                    else:
                        s = j if j <= li else 8 + (j - li - 1)
                        rhs = vv[:, s, :]
                    nc.tensor.matmul(
                        po, E[li][:, j * 128:(j + 1) * 128], rhs,
                        start=(start and j == blocks[0]),
                        stop=(stop and j == last),
                        skip_group_check=True,
                    )
                if stop:
                    nc.gpsimd.tensor_copy(po_sb[:, li, :], po)

            def pv(li):
                pv_mm(li, list(range(2 * li + 2)), True, True)

            # ================= main schedule =================
            # --- own slots 0..7 ---
            kq_own(0, 1, [0, 1, 2, 3], True, False)
            kq_own(0, 1, [4, 5, 6, 7], False, True)
            v_own(0, 1)
            kq_own(1, 1, list(range(8)), True, True)
            v_own(1, 1)
            s_exp(0, [0])
            s_exp(1, [0, 1])
            kq_own(2, 2, list(range(8)), True, True)
            v_own(2, 2)
            s_exp(2, [0, 1, 2])
            s_exp(3, [0, 1, 2, 3])
            kq_own(4, 2, list(range(8)), True, True)
            v_own(4, 2)
            s_exp(4, [0, 1, 2, 3], "a")
            s_exp(4, [4], "b")
            s_exp(5, [0, 1, 2, 3], "a")
            s_exp(5, [4, 5], "b")
            kq_own(6, 2, list(range(8)), True, True)
            v_own(6, 2)
            s_exp(6, [0, 1, 2, 3], "a")
            s_exp(6, [4, 5, 6], "b")
            s_exp(7, [0, 1, 2, 3], "a")
            s_exp(7, [4, 5, 6, 7], "b")
            # --- comp slots 8..15 ---
            kv_comp(8, 2)
            nc.vector.tensor_copy(svec, msk[:, 128:136])  # bf16 -> f32
            tri_mul(0)
            vz_make(0)
            s_exp(0, [1], "c")
            tri_mul(1)
            vz_make(1)
            s_exp(1, [2, 3], "c")
            kv_comp(10, 2)
            pv(0)
            tri_mul(2)
            vz_make(2)
            s_exp(2, [3, 4, 5], "c")
            pv(1)
            nc.sync.dma_start(y_d[:, 0:2, :], po_sb[:, 0:2, :])
            tri_mul(3)
            vz_make(3)
            s_exp(3, [4, 5, 6, 7], "c")
            s_exp(4, [5, 6, 7, 8], "c")
            s_exp(5, [6, 7, 8, 9], "c")
            kv_comp(12, 2)
            pv(2)
            s_exp(6, [7, 8, 9, 10], "c")
            pv(3)
            nc.sync.dma_start(y_d[:, 2:4, :], po_sb[:, 2:4, :])
            s_exp(7, [8, 9, 10, 11], "c")
            tri_mul(4)
            vz_make(4)
            s_exp(4, [9], "d")
            tri_mul(5)
            vz_make(5)
            s_exp(5, [10, 11], "d")
            kv_comp(14, 1)
            pv(4)
            s_exp(6, [11, 12], "d")
            s_exp(7, [12, 13], "d")
            pv(5)
            nc.sync.dma_start(y_d[:, 4:6, :], po_sb[:, 4:6, :])
            tri_mul(6)
            vz_make(6)
            s_exp(6, [13], "e")
            s_exp(7, [14], "e")
            # slot 15 (comp), split across its two half-DMAs
            kv_comp(15, 1, chunks=(0, 1, 2, 3), start=True, stop=False)
            pv(6)
            nc.sync.dma_start(y_d[:, 6:7, :], po_sb[:, 6:7, :])
            kv_comp(15, 1, chunks=(4, 5, 6, 7), start=False, stop=True)
            tri_mul(7)
            vz_make(7)
            pv_mm(7, list(range(15)), True, False)
            s_exp(7, [15], "f")
            pv_mm(7, [15], False, True)
            nc.sync.dma_start(y_d[:, 7:8, :], po_sb[:, 7:8, :])

    nc.compile()
    return nc


def _host_inputs(x, Wq, Wk, Wv):
    """Per-core input maps. Core c = 2*b + jj."""
    x16 = x.astype(np.float16)
    wk16 = Wk.astype(np.float16).reshape(8, 128, DK)
    wq16 = Wq.astype(np.float16).reshape(8, 128, DK)
    wv16 = (Wv / 8.0).astype(np.float16).reshape(8, 128, DK)
    w_h = np.empty((128, 8, 3, DK), dtype=np.float16)
    w_h[:, :, 0, :] = wk16.transpose(1, 0, 2)
    w_h[:, :, 1, :] = wq16.transpose(1, 0, 2)
    w_h[:, :, 2, :] = wv16.transpose(1, 0, 2)
    idt = np.eye(64, dtype=np.float16)
    tri = (np.arange(128)[:, None] <= np.arange(128)[None, :])
    in_maps = []
    for core in range(8):
        b, jj = divmod(core, 2)
        sel = [int(k >= 4) if jj == 0 else int(k < 4) for k in range(8)]
        g = [2 * k + sel[k] for k in range(8)]
        cg = [2 * k + 1 - sel[k] for k in range(8)]
        slot_order = g + cg
        arr = x16[b].reshape(16, 128, 8, 128)         # [tile, r, ch, p]
        xt = np.ascontiguousarray(
            arr[slot_order].transpose(3, 0, 2, 1).reshape(128, NSLOT, 1024))
        msk = np.zeros((128, 136), dtype=np.float32)
        msk[:, 0:128] = tri
        msk[:, 128:136] = np.asarray(sel, dtype=np.float32)
        in_maps.append({
            "xt": xt,
            "w": w_h,
            "idt": idt,
            "msk": msk.astype(ml_dtypes.bfloat16),
        })
    return in_maps


def kernel(x, Wq, Wk, Wv):
    from concourse.bass_utils import run_bass_kernel_spmd

    x = np.asarray(x, dtype=np.float32)
    Wq = np.asarray(Wq, dtype=np.float32)
    Wk = np.asarray(Wk, dtype=np.float32)
    Wv = np.asarray(Wv, dtype=np.float32)

    if "nc" not in _CACHE:
        _CACHE["nc"] = _build()
    nc = _CACHE["nc"]

    in_maps = _host_inputs(x, Wq, Wk, Wv)
    res = run_bass_kernel_spmd(nc, in_maps, core_ids=list(range(8)))
    out = np.empty((B, T, DK), dtype=np.float32)
    for core in range(8):
        b, jj = divmod(core, 2)
        sel = [int(k >= 4) if jj == 0 else int(k < 4) for k in range(8)]
        yloc = res.results[core]["y"]                 # [128, 8, 65]
        for li in range(NLI):
            gt = 2 * li + sel[li]
            out[b, gt * 128:(gt + 1) * 128, :] = (
                yloc[:, li, 0:DK] / yloc[:, li, DK:DK + 1])
    return out


# revision 5
# speedup vs baseline: 1.0368x; 1.0368x over previous
"""Causal single-head attention block on 8 TRN2 NeuronCores.

Reference: Q=x@Wq, K=x@Wk, V=x@Wv; S=Q@K^T (no pre-softmax scaling);
causal mask; P=softmax(S); out=(P@V)/sqrt(64).
Shapes: x [4, 2048, 1024] f32, W* [1024, 64] f32 -> out [4, 2048, 64].

Sharding: 8 cores = 4 batches x 2 interleaved query-tile sets.
Core (b, jj) owns 8 query tiles of 128 rows:
  jj=0: g = {0,2,4,6,9,11,13,15},  jj=1: g = {1,3,5,7,8,10,12,14}
Both sets have equal causal work at 128-key granularity. Host permutes
x[b]^T (fp16) into "slots": 0..7 = own query tiles ascending, 8..15 =
complementary tiles ascending, so the device program is SPMD. Causality
= one shared triangular mask on the diagonal block + a per-core 0/1
scalar folded into a pre-scaled boundary V slot ("vz").

On-chip dataflow per core (x^T resident in SBUF, fp16):
  own slots:  psum[kq, t] = [Wk|Wq]^T @ x^T   (fused, 1 cy/col)
              psum[t, v]  = x^T-tile.T @ (Wv/8)  (natural V)
  comp slots: psum[kv, t] = [Wk|Wv/8]^T @ x^T (fused: V^T rides free)
              V^T -> V via 64-col PE transpose (identity operand)
  S^T tile [t,q] = K^T-slot.T @ Q^T-tile -> exp on ACT -> E^T bf16
  E^T diag block *= tri (DVE, 16-bit 2x mode)
  out psum [q, 65] += E^T-block.T @ [V | 1]  (ones col = row sums)
  raw psum [64 cols + rowsum] shipped to DRAM; host divides.

Engine split: PE matmuls; ACT exp (1-4 block chunks); DVE kt/qt/vt
pair copies + tri + svec; Pool(GpSimd) vv/vz/po copies; SP all DMAs.
Two tiny warmup matmuls set pe_busy_start early (later matmuls are
costed against that timestamp; full clock ~3us after it).
"""

import sys

import numpy as np
import ml_dtypes

try:  # concourse ships in the TRN container; fall back to its known path
    import concourse  # noqa: F401
except ImportError:
    sys.path.insert(0, "/opt/trn_rl_repo")

B, T, C, DK = 4, 2048, 1024, 64
NLI = 8          # query tiles per core
NSLOT = 16       # key tiles (slots) per batch

_CACHE = {}


def _build():
    import concourse.bacc as bacc
    import concourse.tile as tile
    import concourse.mybir as mybir

    f32 = mybir.dt.float32
    f16 = mybir.dt.float16
    bf16 = mybir.dt.bfloat16
    EXP = mybir.ActivationFunctionType.Exp

    nc = bacc.Bacc("TRN2", target_bir_lowering=False, debug=False,
                   enable_asserts=False, num_devices=8)

    xt_d = nc.dram_tensor("xt", [128, NSLOT, 1024], f16,
                          kind="ExternalInput").ap()
    w_d = nc.dram_tensor("w", [128, 8, 3, 64], f16,
                         kind="ExternalInput").ap()
    idt_d = nc.dram_tensor("idt", [64, 64], f16, kind="ExternalInput").ap()
    msk_d = nc.dram_tensor("msk", [128, 136], bf16,
                           kind="ExternalInput").ap()
    y_d = nc.dram_tensor("y", [128, NLI, DK + 1], f32,
                         kind="ExternalOutput").ap()

    with tile.TileContext(nc) as tc:
        with (
            tc.tile_pool(name="persist", bufs=1) as pp,
            tc.tile_pool(name="pmix", bufs=2, space="PSUM") as pmx,
            tc.tile_pool(name="pst", bufs=4, space="PSUM") as pst,
            tc.tile_pool(name="pout", bufs=2, space="PSUM") as pou,
        ):
            xt = pp.tile([128, NSLOT, 1024], f16, tag="xt", name="xt")
            w = pp.tile([128, 8, 3, 64], f16, tag="w", name="w")
            idt = pp.tile([64, 64], f16, tag="idt", name="idt")
            msk = pp.tile([128, 136], bf16, tag="msk", name="msk")
            tri = msk[:, 0:128]
            svec = pp.tile([128, NLI], f32, tag="svec", name="svec")
            kt = pp.tile([64, NSLOT, 128], f16, tag="kt", name="kt")
            qt = pp.tile([64, NLI, 128], f16, tag="qt", name="qt")
            vt = pp.tile([64, NLI, 128], f16, tag="vt", name="vt")
            vv = pp.tile([128, NSLOT, DK + 1], bf16, tag="vv", name="vv")
            vz = pp.tile([128, NLI, DK + 1], bf16, tag="vz", name="vz")
            po_sb = pp.tile([128, NLI, DK + 1], f32, tag="posb", name="posb")
            scr = pp.tile([128, 128], f16, tag="scr", name="scr")
            E = [pp.tile([128, (2 * li + 2) * 128], bf16, tag=f"E{li}",
                         name=f"E{li}") for li in range(NLI)]

            nc.vector.memset(scr, 0.0)
            nc.vector.memset(vv[:, :, DK:DK + 1], 1.0)

            # ---- DMA program (all on SP; transfers serialize in this
            # order on the shared DMA engines) ----
            nc.sync.dma_start(w[:, 0:5], w_d[:, 0:5])
            nc.sync.dma_start(xt[:, 0, 0:512], xt_d[:, 0, 0:512])
            nc.sync.dma_start(w[:, 5:8], w_d[:, 5:8])
            nc.sync.dma_start(xt[:, 0, 512:1024], xt_d[:, 0, 512:1024])
            nc.sync.dma_start(xt[:, 1, :], xt_d[:, 1, :])
            nc.sync.dma_start(xt[:, 2:4, :], xt_d[:, 2:4, :])
            nc.sync.dma_start(xt[:, 4:6, :], xt_d[:, 4:6, :])
            nc.sync.dma_start(msk, msk_d)
            nc.sync.dma_start(idt, idt_d)
            nc.sync.dma_start(xt[:, 6:8, :], xt_d[:, 6:8, :])
            nc.sync.dma_start(xt[:, 8:10, :], xt_d[:, 8:10, :])
            nc.sync.dma_start(xt[:, 10:12, :], xt_d[:, 10:12, :])
            nc.sync.dma_start(xt[:, 12:14, :], xt_d[:, 12:14, :])
            nc.sync.dma_start(xt[:, 14, :], xt_d[:, 14, :])
            nc.sync.dma_start(xt[:, 15, 0:512], xt_d[:, 15, 0:512])
            nc.sync.dma_start(xt[:, 15, 512:1024], xt_d[:, 15, 512:1024])

            # ---- minimal PE warmup (sets pe_busy_start ~1.2us) ----
            for _ in range(2):
                pw = pmx.tile([128, 2, 128], f32, tag="pmix", name="pw")
                nc.tensor.matmul(pw[:, 0, 0:2], scr, scr[:, 0:2],
                                 start=True, stop=True)

            ps_of = {}

            def kq_own(s0, nslots, chunks, start, stop):
                """[Wk|Wq] fused projection for own slots [s0, s0+n)."""
                key = ("kq", s0)
                if key not in ps_of:
                    ps_of[key] = pmx.tile([128, 2, 128], f32, tag="pmix",
                                          name=f"pkq{s0}")
                ps = ps_of[key]
                for i in range(nslots):
                    s = s0 + i
                    for ch in chunks:
                        nc.tensor.matmul(
                            ps[:, i, :],
                            w[:, ch, 0:2, :],
                            xt[:, s, ch * 128:(ch + 1) * 128],
                            start=(start and ch == chunks[0]),
                            stop=(stop and ch == chunks[-1]),
                        )
                if stop:
                    nc.vector.tensor_copy(qt[:, s0:s0 + nslots, :],
                                          ps[0:64, 0:nslots, :])
                    nc.vector.tensor_copy(kt[:, s0:s0 + nslots, :],
                                          ps[64:128, 0:nslots, :])

            def v_own(s0, nslots):
                """Natural V (pre-scaled 1/8) for own slots [s0, s0+n)."""
                ps = pmx.tile([128, 2, 128], f32, tag="pmix", name=f"pv{s0}")
                for i in range(nslots):
                    s = s0 + i
                    for ch in range(8):
                        nc.tensor.matmul(
                            ps[:, i, 0:DK],
                            xt[:, s, ch * 128:(ch + 1) * 128],
                            w[:, ch, 2, :],
                            start=(ch == 0), stop=(ch == 7),
                        )
                nc.vector.tensor_copy(vv[:, s0:s0 + nslots, 0:DK],
                                      ps[:, 0:nslots, 0:DK])

            def kv_comp(s0, nslots, chunks=tuple(range(8)), start=True,
                        stop=True):
                """[Wk|Wv/8] fused projection for comp slots [s0, s0+n);
                V^T -> vt staging -> PE transpose -> vv natural."""
                key = ("kv", s0)
                if key not in ps_of:
                    ps_of[key] = pmx.tile([128, 2, 128], f32, tag="pmix",
                                          name=f"pkv{s0}")
                ps = ps_of[key]
                for i in range(nslots):
                    s = s0 + i
                    for ch in chunks:
                        nc.tensor.matmul(
                            ps[:, i, :],
                            w[:, ch, 1:3, :],
                            xt[:, s, ch * 128:(ch + 1) * 128],
                            start=(start and ch == chunks[0]),
                            stop=(stop and ch == chunks[-1]),
                        )
                if not stop:
                    return
                k0 = s0 - 8
                nc.vector.tensor_copy(kt[:, s0:s0 + nslots, :],
                                      ps[0:64, 0:nslots, :])
                nc.vector.tensor_copy(vt[:, k0:k0 + nslots, :],
                                      ps[64:128, 0:nslots, :])
                pt = pmx.tile([128, 2, DK], f16, tag="pmix", name=f"pt{s0}")
                for i in range(nslots):
                    nc.tensor.transpose(pt[:, i, :], vt[:, k0 + i, :], idt)
                nc.vector.tensor_copy(vv[:, s0:s0 + nslots, 0:DK],
                                      pt[:, 0:nslots, :])

            def s_exp(li, blocks, tag=""):
                """S^T then exp for E[li] col blocks `blocks` (block j:
                key slot j if j<=li else 8+(j-li-1); q = slot li)."""
                nb = len(blocks)
                ps = pst.tile([128, 512], f32, tag="pst",
                              name=f"ps{li}{tag}")
                for i, j in enumerate(blocks):
                    s = j if j <= li else 8 + (j - li - 1)
                    nc.tensor.matmul(
                        ps[:, i * 128:(i + 1) * 128],
                        kt[:, s, :],
                        qt[:, li, :],
                        start=True, stop=True,
                    )
                j0 = blocks[0]
                nc.scalar.activation(
                    E[li][:, j0 * 128:(j0 + nb) * 128], ps[:, 0:nb * 128],
                    EXP)

            def tri_mul(li):
                # diagonal block (j = li): shared triangular mask on Pool
                nc.gpsimd.tensor_mul(
                    E[li][:, li * 128:(li + 1) * 128],
                    E[li][:, li * 128:(li + 1) * 128], tri)

            def vz_make(li):
                # boundary V slot pre-scaled by the per-core 0/1 scalar
                nc.gpsimd.tensor_scalar_mul(
                    vz[:, li, :], vv[:, 8 + li, :], svec[:, li:li + 1])

            po_t = {}

            def pv_mm(li, blocks, start, stop):
                if li not in po_t:
                    po_t[li] = pou.tile([128, DK + 1], f32, tag="pout",
                                        name=f"po{li}")
                po = po_t[li]
                last = blocks[-1]
                for j in blocks:
                    if j == 2 * li + 1:
                        rhs = vz[:, li, :]
                    else:
                        s = j if j <= li else 8 + (j - li - 1)
                        rhs = vv[:, s, :]
                    nc.tensor.matmul(
                        po, E[li][:, j * 128:(j + 1) * 128], rhs,
                        start=(start and j == blocks[0]),
                        stop=(stop and j == last),
                        skip_group_check=True,
                    )
                if stop:
                    nc.vector.tensor_copy(po_sb[:, li, :], po)

            def pv(li):
                pv_mm(li, list(range(2 * li + 2)), True, True)

            # ================= main schedule =================
            # --- own slots 0..7 ---
            kq_own(0, 1, [0, 1, 2, 3], True, False)
            kq_own(0, 1, [4, 5, 6, 7], False, True)
            v_own(0, 1)
            kq_own(1, 1, list(range(8)), True, True)
            v_own(1, 1)
            s_exp(0, [0])
            s_exp(1, [0, 1])
            kq_own(2, 2, list(range(8)), True, True)
            v_own(2, 2)
            s_exp(2, [0, 1, 2])
            s_exp(3, [0, 1, 2, 3])
            kq_own(4, 2, list(range(8)), True, True)
            v_own(4, 2)
            s_exp(4, [0, 1, 2, 3], "a")
            s_exp(4, [4], "b")
            s_exp(5, [0, 1, 2, 3], "a")
            s_exp(5, [4, 5], "b")
            kq_own(6, 2, list(range(8)), True, True)
            v_own(6, 2)
            s_exp(6, [0, 1, 2, 3], "a")
            s_exp(6, [4, 5, 6], "b")
            s_exp(7, [0, 1, 2, 3], "a")
            s_exp(7, [4, 5, 6, 7], "b")
            # --- comp slots 8..15 ---
            kv_comp(8, 2)
            nc.vector.tensor_copy(svec, msk[:, 128:136])  # bf16 -> f32
            tri_mul(0)
            vz_make(0)
            s_exp(0, [1], "c")
            tri_mul(1)
            vz_make(1)
            s_exp(1, [2, 3], "c")
            kv_comp(10, 2)
            pv(0)
            tri_mul(2)
            vz_make(2)
            s_exp(2, [3, 4, 5], "c")
            pv(1)
            nc.sync.dma_start(y_d[:, 0:2, :], po_sb[:, 0:2, :])
            tri_mul(3)
            vz_make(3)
            s_exp(3, [4, 5, 6, 7], "c")
            s_exp(4, [5, 6, 7, 8], "c")
            s_exp(5, [6, 7, 8, 9], "c")
            kv_comp(12, 2)
            pv(2)
            s_exp(6, [7, 8, 9, 10], "c")
            pv(3)
            nc.sync.dma_start(y_d[:, 2:4, :], po_sb[:, 2:4, :])
            s_exp(7, [8, 9, 10, 11], "c")
            tri_mul(4)
            vz_make(4)
            s_exp(4, [9], "d")
            tri_mul(5)
            vz_make(5)
            s_exp(5, [10, 11], "d")
            kv_comp(14, 1)
            pv(4)
            s_exp(6, [11, 12], "d")
            s_exp(7, [12, 13], "d")
            pv(5)
            nc.sync.dma_start(y_d[:, 4:6, :], po_sb[:, 4:6, :])
            tri_mul(6)
            vz_make(6)
            s_exp(6, [13], "e")
            s_exp(7, [14], "e")
            # slot 15 (comp), split across its two half-DMAs
            kv_comp(15, 1, chunks=(0, 1, 2, 3), start=True, stop=False)
            pv(6)
            nc.sync.dma_start(y_d[:, 6:7, :], po_sb[:, 6:7, :])
            kv_comp(15, 1, chunks=(4, 5, 6, 7), start=False, stop=True)
            tri_mul(7)
            vz_make(7)
            pv_mm(7, list(range(15)), True, False)
            s_exp(7, [15], "f")
            pv_mm(7, [15], False, True)
            nc.sync.dma_start(y_d[:, 7:8, :], po_sb[:, 7:8, :])

    nc.compile()
    return nc


def _host_inputs(x, Wq, Wk, Wv):
    """Per-core input maps. Core c = 2*b + jj."""
    x16 = x.astype(np.float16)
    wk16 = Wk.astype(np.float16).reshape(8, 128, DK)
    wq16 = Wq.astype(np.float16).reshape(8, 128, DK)
    wv16 = (Wv / 8.0).astype(np.float16).reshape(8, 128, DK)
    w_h = np.empty((128, 8, 3, DK), dtype=np.float16)
    w_h[:, :, 0, :] = wq16.transpose(1, 0, 2)
    w_h[:, :, 1, :] = wk16.transpose(1, 0, 2)
    w_h[:, :, 2, :] = wv16.transpose(1, 0, 2)
    idt = np.eye(64, dtype=np.float16)
    tri = (np.arange(128)[:, None] <= np.arange(128)[None, :])
    in_maps = []
    for core in range(8):
        b, jj = divmod(core, 2)
        sel = [int(k >= 4) if jj == 0 else int(k < 4) for k in range(8)]
        g = [2 * k + sel[k] for k in range(8)]
        cg = [2 * k + 1 - sel[k] for k in range(8)]
        slot_order = g + cg
        arr = x16[b].reshape(16, 128, 8, 128)         # [tile, r, ch, p]
        xt = np.ascontiguousarray(
            arr[slot_order].transpose(3, 0, 2, 1).reshape(128, NSLOT, 1024))
        msk = np.zeros((128, 136), dtype=np.float32)
        msk[:, 0:128] = tri
        msk[:, 128:136] = np.asarray(sel, dtype=np.float32)
        in_maps.append({
            "xt": xt,
            "w": w_h,
            "idt": idt,
            "msk": msk.astype(ml_dtypes.bfloat16),
        })
    return in_maps


def kernel(x, Wq, Wk, Wv):
    from concourse.bass_utils import run_bass_kernel_spmd

    x = np.asarray(x, dtype=np.float32)
    Wq = np.asarray(Wq, dtype=np.float32)
    Wk = np.asarray(Wk, dtype=np.float32)
    Wv = np.asarray(Wv, dtype=np.float32)

    if "nc" not in _CACHE:
        _CACHE["nc"] = _build()
    nc = _CACHE["nc"]

    in_maps = _host_inputs(x, Wq, Wk, Wv)
    res = run_bass_kernel_spmd(nc, in_maps, core_ids=list(range(8)))
    out = np.empty((B, T, DK), dtype=np.float32)
    for core in range(8):
        b, jj = divmod(core, 2)
        sel = [int(k >= 4) if jj == 0 else int(k < 4) for k in range(8)]
        yloc = res.results[core]["y"]                 # [128, 8, 65]
        for li in range(NLI):
            gt = 2 * li + sel[li]
            out[b, gt * 128:(gt + 1) * 128, :] = (
                yloc[:, li, 0:DK] / yloc[:, li, DK:DK + 1])
    return out
